# revision 8
# baseline (speedup 1.0000x reference)
"""BSQ (binary spherical quantization) forward kernel for Trainium2, 8 cores.

Math: recon = sign(x @ Wp + bp) @ Wr + br, with sign(v) = 2*(v>=0)-1.
The L2-normalize in the reference doesn't change signs, so it drops out.

Device-side formulation (per 128-token chunk):
  xT   = PE-transpose(x)                      (exact f32 permutation)
  zT   = Wp.T @ xT                            (f32 matmul, PSUM accumulate over C)
  b    = (zT >= -bp)  in {0,1}                (DVE is_ge, exact reference semantics)
  recon = [b; 1].T @ [2*Wr; br - sum(Wr)]     (f32r matmul; {0,1} exact in f32r,
                                               weight rounding ~1e-4 relative)

Sharding: pure data parallel over the 65536 tokens, 8192 tokens per core.
"""

import numpy as np
import concourse.bacc as bacc
import concourse.tile as tile
import concourse.mybir as mybir
import concourse.bass as bass
from concourse.bass_utils import run_bass_kernel_spmd

N_CORES = 8
B, H, W_, C = 64, 32, 32, 512
BITS = 16
T_TOTAL = B * H * W_          # 65536 tokens
T_CORE = T_TOTAL // N_CORES   # 8192 tokens per core
GROUP = 512                   # tokens per compute group
SUPER = 2048                  # tokens per input-DMA super group
G_PER_S = SUPER // GROUP      # 4
NC_CHUNKS = C // 128          # 4 C-chunks of 128
NT_CHUNKS = GROUP // 128      # 4 token chunks of 128
N_WARMUP = 40                 # bf16 dummy matmuls to warm the PE clock gate

F32 = mybir.dt.float32
F32R = mybir.dt.float32r
BF16 = mybir.dt.bfloat16


def build_nc(t_core=T_CORE):
    n_supers = t_core // SUPER
    nc = bacc.Bacc("TRN2", debug=False, enable_asserts=False, num_devices=N_CORES)

    x_d = nc.dram_tensor("x", [t_core, C], F32, kind="ExternalInput").ap()
    wp_d = nc.dram_tensor("wp", [128, NC_CHUNKS * BITS], F32, kind="ExternalInput").ap()
    wr2e_d = nc.dram_tensor("wr2e", [BITS + 1, C], F32, kind="ExternalInput").ap()
    negbp_d = nc.dram_tensor("negbp", [BITS, 1], F32, kind="ExternalInput").ap()
    id_d = nc.dram_tensor("ident", [128, 128], F32, kind="ExternalInput").ap()
    out_d = nc.dram_tensor("out", [t_core, C], F32, kind="ExternalOutput").ap()

    with tile.TileContext(nc) as tc:
        with tc.tile_pool(name="const", bufs=1) as constp, \
             tc.tile_pool(name="xin", bufs=2) as xin_p, \
             tc.tile_pool(name="xt", bufs=8) as xt_p, \
             tc.tile_pool(name="rout", bufs=2) as rout_p, \
             tc.tile_pool(name="ps_xt", bufs=4, space="PSUM") as ps_xt_p, \
             tc.tile_pool(name="ps_z", bufs=2, space="PSUM") as ps_z_p, \
             tc.tile_pool(name="ps_r", bufs=2, space="PSUM") as ps_r_p:

            ident = constp.tile([128, 128], F32, tag="ident")
            wp = constp.tile([128, NC_CHUNKS * BITS], F32, tag="wp")
            wr2e_stage = constp.tile([BITS + 1, C], F32, tag="wr2s")
            wr2e = constp.tile([BITS + 1, C], F32R, tag="wr2r")
            negbp = constp.tile([BITS, 1], F32, tag="negbp")
            nc.sync.dma_start(ident[:], id_d[:])
            nc.sync.dma_start(wp[:], wp_d[:])
            nc.sync.dma_start(wr2e_stage[:], wr2e_d[:])
            nc.sync.dma_start(negbp[:], negbp_d[:])
            # round the mm2 weights into f32r once
            nc.vector.tensor_copy(wr2e[:], wr2e_stage[:])

            # PE clock-gate warmup: dense bf16 matmul burst while the first
            # input DMA is in flight (the HAM needs ~3.4us of busy to unthrottle)
            wu = constp.tile([128, 640], BF16, tag="wu")
            nc.vector.memset(wu[:], 0.0)
            wu_ps = ps_r_p.tile([128, C], F32, tag="rps", name="wu_ps")
            for i in range(N_WARMUP):
                nc.tensor.matmul(wu_ps[:], wu[:, 0:128], wu[:, 128:640],
                                 start=True, stop=True)

            # two persistent code tiles (even/odd groups); ones row written once
            codeT = [constp.tile([BITS + 1, GROUP], F32R, tag=f"code{i}",
                                 name=f"codeT{i}")
                     for i in range(2)]
            for ct in codeT:
                nc.vector.memset(ct[:].bitcast(F32), 1.0)

            gidx = 0
            for s in range(n_supers):
                x_sb = xin_p.tile([128, (SUPER // 128) * C], F32, tag="x")
                src = x_d[s * SUPER:(s + 1) * SUPER, :].rearrange(
                    "(t p) c -> p t c", p=128)
                nc.sync.dma_start(x_sb[:].rearrange("p (t c) -> p t c", c=C), src)
                recon_sb = rout_p.tile([128, (SUPER // 128) * C], F32, tag="r")

                for g in range(G_PER_S):
                    t0 = g * NT_CHUNKS
                    # transpose x into [C, T] chunks, then PSUM -> SBUF
                    xt_sbs = []
                    for c in range(NC_CHUNKS):
                        xt_ps = ps_xt_p.tile([128, GROUP], F32, tag="xtps")
                        for t in range(NT_CHUNKS):
                            nc.tensor.transpose(
                                xt_ps[:, t * 128:(t + 1) * 128],
                                x_sb[:, (t0 + t) * C + c * 128:
                                     (t0 + t) * C + (c + 1) * 128],
                                ident[:])
                        xt_sb = xt_p.tile([128, GROUP], F32, tag="xts")
                        if c % 2 == 0:
                            nc.scalar.copy(xt_sb[:], xt_ps[:])
                        else:
                            nc.vector.tensor_copy(xt_sb[:], xt_ps[:])
                        xt_sbs.append(xt_sb)

                    # mm1: zT accumulated over the 4 C-chunks
                    zT_ps = ps_z_p.tile([BITS, GROUP], F32, tag="zt")
                    for c in range(NC_CHUNKS):
                        nc.tensor.matmul(zT_ps[:], wp[:, c * BITS:(c + 1) * BITS],
                                         xt_sbs[c][:],
                                         start=(c == 0), stop=(c == NC_CHUNKS - 1))

                    # codes b = (zT >= -bp)
                    ct = codeT[gidx % 2]
                    nc.vector.tensor_scalar(ct[0:BITS, :], zT_ps[:], negbp[:], None,
                                            op0=mybir.AluOpType.is_ge)

                    # mm2 + bias via the ones row; f32r at full PE rate
                    for t in range(NT_CHUNKS):
                        r_ps = ps_r_p.tile([128, C], F32, tag="rps")
                        nc.tensor.matmul(r_ps[:], ct[:, t * 128:(t + 1) * 128],
                                         wr2e[:], start=True, stop=True)
                        dst_sl = recon_sb[:, (t0 + t) * C:(t0 + t + 1) * C]
                        if t % 2 == 0:
                            nc.scalar.copy(dst_sl, r_ps[:])
                        else:
                            nc.vector.tensor_copy(dst_sl, r_ps[:])

                    # per-group 1 MB store on the scalar HWDGE ring
                    dst = out_d[s * SUPER + g * GROUP:
                                s * SUPER + (g + 1) * GROUP, :].rearrange(
                        "(t p) c -> p t c", p=128)
                    nc.scalar.dma_start(
                        dst, recon_sb[:, t0 * C:(t0 + NT_CHUNKS) * C].rearrange(
                            "p (t c) -> p t c", c=C))
                    gidx += 1

    nc.compile()
    return nc


def _execute(in_maps):
    nc = build_nc()
    res = run_bass_kernel_spmd(nc, in_maps, core_ids=list(range(N_CORES)))
    return np.concatenate([res.results[i]["out"] for i in range(N_CORES)], axis=0)


def _subprocess_worker(in_maps, q):
    try:
        q.put(("ok", _execute(in_maps)))
    except Exception as e:  # noqa: BLE001
        q.put(("err", repr(e)))


def _execute_with_retry(in_maps, attempts=3):
    """The first device execution in a process occasionally dies with
    NRT_EXEC_UNIT_UNRECOVERABLE (infra flake); the jax client is then
    poisoned, so retries run in a fresh subprocess."""
    try:
        return _execute(in_maps)
    except Exception:  # noqa: BLE001
        import multiprocessing as mp
        last = None
        for _ in range(attempts):
            ctx = mp.get_context("spawn")
            q = ctx.Queue()
            p = ctx.Process(target=_subprocess_worker, args=(in_maps, q))
            p.start()
            try:
                status, payload = q.get(timeout=1800)
            except Exception as e:  # noqa: BLE001
                status, payload = "err", repr(e)
            p.join(timeout=60)
            if p.is_alive():
                p.terminate()
            if status == "ok":
                return payload
            last = payload
        raise RuntimeError(f"kernel execution failed after retries: {last}")


def kernel(x, Wp, bp, Wr, br):
    x = np.asarray(x, dtype=np.float32)
    Wp = np.asarray(Wp, dtype=np.float32)
    bp = np.asarray(bp, dtype=np.float32)
    Wr = np.asarray(Wr, dtype=np.float32)
    br = np.asarray(br, dtype=np.float32)
    assert x.shape == (B, H, W_, C), x.shape

    xf = np.ascontiguousarray(x.reshape(T_TOTAL, C))
    # Wp [C, BITS] -> [128, 4*BITS], chunk c of the contraction at cols c*16..
    wp_r = np.ascontiguousarray(
        Wp.reshape(NC_CHUNKS, 128, BITS).transpose(1, 0, 2).reshape(128, NC_CHUNKS * BITS))
    wr2e = np.ascontiguousarray(
        np.concatenate([2.0 * Wr, (br - Wr.sum(axis=0))[None, :]], axis=0))
    negbp = np.ascontiguousarray((-bp).reshape(BITS, 1))
    ident = np.eye(128, dtype=np.float32)

    in_maps = []
    for i in range(N_CORES):
        in_maps.append({
            "x": np.ascontiguousarray(xf[i * T_CORE:(i + 1) * T_CORE]),
            "wp": wp_r, "wr2e": wr2e, "negbp": negbp, "ident": ident,
        })

    out = _execute_with_retry(in_maps)
    return out.reshape(B, H, W_, C)


# revision 10
# speedup vs baseline: 1.0559x; 1.0559x over previous
"""BSQ (binary spherical quantization) forward kernel for Trainium2, 8 cores.

Math: recon = sign(x @ Wp + bp) @ Wr + br, with sign(v) = 2*(v>=0)-1.
The L2-normalize in the reference doesn't change signs, so it drops out.

Device-side formulation (per 128-token chunk):
  xT   = PE-transpose(x)                      (exact f32 permutation)
  zT   = Wp.T @ xT                            (f32 matmul, PSUM accumulate over C)
  b    = (zT >= -bp)  in {0,1}                (DVE is_ge, exact reference semantics)
  recon = [b; 1].T @ [2*Wr; br - sum(Wr)]     (f32r matmul; {0,1} exact in f32r,
                                               weight rounding ~1e-4 relative)

Sharding: pure data parallel over the 65536 tokens, 8192 tokens per core.
"""

import numpy as np
import concourse.bacc as bacc
import concourse.tile as tile
import concourse.mybir as mybir
import concourse.bass as bass
from concourse.bass_utils import run_bass_kernel_spmd

N_CORES = 8
B, H, W_, C = 64, 32, 32, 512
BITS = 16
T_TOTAL = B * H * W_          # 65536 tokens
T_CORE = T_TOTAL // N_CORES   # 8192 tokens per core
GROUP = 512                   # tokens per compute group
SUPER = 2048                  # tokens per input-DMA super group
G_PER_S = SUPER // GROUP      # 4
NC_CHUNKS = C // 128          # 4 C-chunks of 128
NT_CHUNKS = GROUP // 128      # 4 token chunks of 128
N_WARMUP = 24                 # bf16 dummy matmuls to warm the PE clock gate

F32 = mybir.dt.float32
F32R = mybir.dt.float32r
BF16 = mybir.dt.bfloat16


def build_nc(t_core=T_CORE):
    n_supers = t_core // SUPER
    nc = bacc.Bacc("TRN2", debug=False, enable_asserts=False, num_devices=N_CORES)

    x_d = nc.dram_tensor("x", [t_core, C], F32, kind="ExternalInput").ap()
    wp_d = nc.dram_tensor("wp", [128, NC_CHUNKS * BITS], F32, kind="ExternalInput").ap()
    wr2e_d = nc.dram_tensor("wr2e", [BITS + 1, C], F32, kind="ExternalInput").ap()
    negbp_d = nc.dram_tensor("negbp", [BITS, 1], F32, kind="ExternalInput").ap()
    id_d = nc.dram_tensor("ident", [128, 128], F32, kind="ExternalInput").ap()
    out_d = nc.dram_tensor("out", [t_core, C], F32, kind="ExternalOutput").ap()

    with tile.TileContext(nc) as tc:
        with tc.tile_pool(name="const", bufs=1) as constp, \
             tc.tile_pool(name="xin", bufs=2) as xin_p, \
             tc.tile_pool(name="xt", bufs=8) as xt_p, \
             tc.tile_pool(name="rout", bufs=2) as rout_p, \
             tc.tile_pool(name="ps_xt", bufs=4, space="PSUM") as ps_xt_p, \
             tc.tile_pool(name="ps_z", bufs=2, space="PSUM") as ps_z_p, \
             tc.tile_pool(name="ps_r", bufs=2, space="PSUM") as ps_r_p:

            ident = constp.tile([128, 128], F32, tag="ident")
            wp = constp.tile([128, NC_CHUNKS * BITS], F32, tag="wp")
            wr2e_stage = constp.tile([BITS + 1, C], F32, tag="wr2s")
            wr2e = constp.tile([BITS + 1, C], F32R, tag="wr2r")
            negbp = constp.tile([BITS, 1], F32, tag="negbp")
            nc.sync.dma_start(ident[:], id_d[:])
            nc.sync.dma_start(wp[:], wp_d[:])
            nc.sync.dma_start(wr2e_stage[:], wr2e_d[:])
            nc.sync.dma_start(negbp[:], negbp_d[:])
            # round the mm2 weights into f32r once
            nc.vector.tensor_copy(wr2e[:], wr2e_stage[:])

            # PE clock-gate warmup: dense bf16 matmul burst while the first
            # input DMA is in flight (the HAM needs ~3.4us of busy to unthrottle)
            wu = constp.tile([128, 640], BF16, tag="wu")
            nc.vector.memset(wu[:], 0.0)
            wu_ps = ps_r_p.tile([128, C], F32, tag="rps", name="wu_ps")
            for i in range(N_WARMUP):
                nc.tensor.matmul(wu_ps[:], wu[:, 0:128], wu[:, 128:640],
                                 start=True, stop=True)

            # two persistent code tiles (even/odd groups); ones row written once
            codeT = [constp.tile([BITS + 1, GROUP], F32R, tag=f"code{i}",
                                 name=f"codeT{i}")
                     for i in range(2)]
            for ct in codeT:
                nc.vector.memset(ct[:].bitcast(F32), 1.0)

            gidx = 0
            for s in range(n_supers):
                x_sb = xin_p.tile([128, (SUPER // 128) * C], F32, tag="x")
                if s == 0:
                    # 1 MB-granular loads so group 0 can start ~4us in;
                    # later supers use single 4 MB loads for peak DMA rate
                    for g in range(G_PER_S):
                        src = x_d[g * GROUP:(g + 1) * GROUP, :].rearrange(
                            "(t p) c -> p t c", p=128)
                        nc.sync.dma_start(
                            x_sb[:, g * NT_CHUNKS * C:(g + 1) * NT_CHUNKS * C]
                            .rearrange("p (t c) -> p t c", c=C), src)
                else:
                    src = x_d[s * SUPER:(s + 1) * SUPER, :].rearrange(
                        "(t p) c -> p t c", p=128)
                    nc.sync.dma_start(x_sb[:].rearrange("p (t c) -> p t c", c=C), src)
                recon_sb = rout_p.tile([128, (SUPER // 128) * C], F32, tag="r")

                for g in range(G_PER_S):
                    t0 = g * NT_CHUNKS
                    # transpose x into [C, T] chunks, then PSUM -> SBUF
                    xt_sbs = []
                    for c in range(NC_CHUNKS):
                        xt_ps = ps_xt_p.tile([128, GROUP], F32, tag="xtps")
                        for t in range(NT_CHUNKS):
                            nc.tensor.transpose(
                                xt_ps[:, t * 128:(t + 1) * 128],
                                x_sb[:, (t0 + t) * C + c * 128:
                                     (t0 + t) * C + (c + 1) * 128],
                                ident[:])
                        xt_sb = xt_p.tile([128, GROUP], F32, tag="xts")
                        if c % 2 == 0:
                            nc.scalar.copy(xt_sb[:], xt_ps[:])
                        else:
                            nc.vector.tensor_copy(xt_sb[:], xt_ps[:])
                        xt_sbs.append(xt_sb)

                    # mm1: zT accumulated over the 4 C-chunks
                    zT_ps = ps_z_p.tile([BITS, GROUP], F32, tag="zt")
                    for c in range(NC_CHUNKS):
                        nc.tensor.matmul(zT_ps[:], wp[:, c * BITS:(c + 1) * BITS],
                                         xt_sbs[c][:],
                                         start=(c == 0), stop=(c == NC_CHUNKS - 1))

                    # codes b = (zT >= -bp)
                    ct = codeT[gidx % 2]
                    nc.vector.tensor_scalar(ct[0:BITS, :], zT_ps[:], negbp[:], None,
                                            op0=mybir.AluOpType.is_ge)

                    # mm2 + bias via the ones row; f32r at full PE rate
                    for t in range(NT_CHUNKS):
                        r_ps = ps_r_p.tile([128, C], F32, tag="rps")
                        nc.tensor.matmul(r_ps[:], ct[:, t * 128:(t + 1) * 128],
                                         wr2e[:], start=True, stop=True)
                        dst_sl = recon_sb[:, (t0 + t) * C:(t0 + t + 1) * C]
                        if t % 2 == 0:
                            nc.scalar.copy(dst_sl, r_ps[:])
                        else:
                            nc.vector.tensor_copy(dst_sl, r_ps[:])

                    # per-group 1 MB store on the scalar HWDGE ring
                    dst = out_d[s * SUPER + g * GROUP:
                                s * SUPER + (g + 1) * GROUP, :].rearrange(
                        "(t p) c -> p t c", p=128)
                    nc.scalar.dma_start(
                        dst, recon_sb[:, t0 * C:(t0 + NT_CHUNKS) * C].rearrange(
                            "p (t c) -> p t c", c=C))
                    gidx += 1

    nc.compile()
    return nc


def _execute(in_maps):
    nc = build_nc()
    res = run_bass_kernel_spmd(nc, in_maps, core_ids=list(range(N_CORES)))
    return np.concatenate([res.results[i]["out"] for i in range(N_CORES)], axis=0)


def _subprocess_worker(in_maps, q):
    try:
        q.put(("ok", _execute(in_maps)))
    except Exception as e:  # noqa: BLE001
        q.put(("err", repr(e)))


def _execute_with_retry(in_maps, attempts=3):
    """The first device execution in a process occasionally dies with
    NRT_EXEC_UNIT_UNRECOVERABLE (infra flake); the jax client is then
    poisoned, so retries run in a fresh subprocess."""
    try:
        return _execute(in_maps)
    except Exception:  # noqa: BLE001
        import multiprocessing as mp
        last = None
        for _ in range(attempts):
            ctx = mp.get_context("spawn")
            q = ctx.Queue()
            p = ctx.Process(target=_subprocess_worker, args=(in_maps, q))
            p.start()
            try:
                status, payload = q.get(timeout=1800)
            except Exception as e:  # noqa: BLE001
                status, payload = "err", repr(e)
            p.join(timeout=60)
            if p.is_alive():
                p.terminate()
            if status == "ok":
                return payload
            last = payload
        raise RuntimeError(f"kernel execution failed after retries: {last}")


def kernel(x, Wp, bp, Wr, br):
    x = np.asarray(x, dtype=np.float32)
    Wp = np.asarray(Wp, dtype=np.float32)
    bp = np.asarray(bp, dtype=np.float32)
    Wr = np.asarray(Wr, dtype=np.float32)
    br = np.asarray(br, dtype=np.float32)
    assert x.shape == (B, H, W_, C), x.shape

    xf = np.ascontiguousarray(x.reshape(T_TOTAL, C))
    # Wp [C, BITS] -> [128, 4*BITS], chunk c of the contraction at cols c*16..
    wp_r = np.ascontiguousarray(
        Wp.reshape(NC_CHUNKS, 128, BITS).transpose(1, 0, 2).reshape(128, NC_CHUNKS * BITS))
    wr2e = np.ascontiguousarray(
        np.concatenate([2.0 * Wr, (br - Wr.sum(axis=0))[None, :]], axis=0))
    negbp = np.ascontiguousarray((-bp).reshape(BITS, 1))
    ident = np.eye(128, dtype=np.float32)

    in_maps = []
    for i in range(N_CORES):
        in_maps.append({
            "x": np.ascontiguousarray(xf[i * T_CORE:(i + 1) * T_CORE]),
            "wp": wp_r, "wr2e": wr2e, "negbp": negbp, "ident": ident,
        })

    out = _execute_with_retry(in_maps)
    return out.reshape(B, H, W_, C)
